# revision 26
# baseline (speedup 1.0000x reference)
"""Trainium2 Bass kernel for nn_AttentionTSSA (B=8, N=8192, C=512, H=8).

Sharding: data-parallel over batch B across the 8 NeuronCores (1 batch each,
no collectives).  bf16 data path throughout (inputs/outputs quantized on the
host); all matmul accumulation and the norm/softmax reductions stay f32.

Per core, three PE-paced stages with engine-balanced helpers:

  stage 1: wT[c,n] = Wqkv @ x^T in 512-token chunks (bf16 matmuls, f32 PSUM).
           Act evacuates PSUM -> wt (bf16); DVE tensor_tensor_reduce squares
           the same PSUM tile into w2 (bf16) AND emits the per-channel
           norm^2 partial in one pass.  DMA-in (bf16) on the SP queue.
  stage 2: per 128-token tile, a [c,16]-column matmul with the w2 tile as
           stationary produces (sum_ws | r) token-major.  Act computes
           exp(mask*logits + mbias) straight from PSUM (mask folded via
           scale/bias APs), DVE finishes the head-softmax; Pi and Pi*r are
           accumulated into the global S/PR bank with ones-matmuls.  Pi is
           transposed (PE) and broadcast to channel layout with a 0-stride
           SBUF DMA, then DVE scales wt in place (all bf16).
  stage 3: outT = Wout_scaled.T @ wt_scaled as a pure GEMM stream; -1/(1+dots)
           is folded into the wo weights.  Act/DVE alternate on the biased
           PSUM evacuation; DMA-out (bf16) on SP.

Host side: x is transposed/cast to bf16 per batch; outputs are cast back and
un-transposed.
"""

import numpy as np

B, N, C, H = 8, 8192, 512, 8
D = C // H          # 64
CT = C // 128       # 4 channel tiles
NCH = N // 512      # 16 chunks of 512 tokens
TPC = 4             # token tiles per chunk
NT = N // 128       # 64 token tiles

_CACHE = {}


def _build_bass(reps=1, phases=(1, 2, 3)):
    import concourse.bacc as bacc
    import concourse.bass as bass
    import concourse.mybir as mybir
    import concourse.tile as tile

    f32 = mybir.dt.float32
    bf16 = mybir.dt.bfloat16
    Alu = mybir.AluOpType
    Act = mybir.ActivationFunctionType

    nc = bacc.Bacc("TRN2", target_bir_lowering=False, debug=False, num_devices=B)

    xbf = nc.dram_tensor("xbf", [C, N], bf16, kind="ExternalInput")
    wqkvb = nc.dram_tensor("wqkvb", [C, C], bf16, kind="ExternalInput")
    woutb = nc.dram_tensor("woutb", [C, C], bf16, kind="ExternalInput")
    boutT = nc.dram_tensor("boutT", [128, CT], f32, kind="ExternalInput")
    maskf = nc.dram_tensor("maskf", [128, NT], f32, kind="ExternalInput")
    m8q = nc.dram_tensor("m8q", [128, NT], f32, kind="ExternalInput")
    tempP = nc.dram_tensor("tempP", [128, CT], f32, kind="ExternalInput")
    identb = nc.dram_tensor("identb", [128, 128], bf16, kind="ExternalInput")
    amatc = nc.dram_tensor("amatc", [128, CT * 16], bf16, kind="ExternalInput")
    outT = nc.dram_tensor("outT", [C, N], bf16, kind="ExternalOutput")

    with tile.TileContext(nc) as tc:
        with (
            tc.tile_pool(name="singles", bufs=1) as sing,
            tc.tile_pool(name="small", bufs=2) as small,
        ):
            # ---------------- constants / persistent tiles ----------------
            wq = [sing.tile([128, C], bf16, tag=f"wq{i}", name=f"wq{i}") for i in range(CT)]
            wo = [sing.tile([128, C], bf16, tag=f"wo{i}", name=f"wo{i}") for i in range(CT)]
            wt = [sing.tile([128, N], bf16, tag=f"wt{i}", name=f"wt{i}") for i in range(CT)]
            w2 = [sing.tile([128, N], bf16, tag=f"w2_{i}", name=f"w2_{i}") for i in range(CT)]
            # wq loads are interleaved with the first x chunk inside phase1
            # (SP queue); everything else loads via the Activation DMA queue.
            for i in range(CT):
                nc.scalar.dma_start(out=wo[i][:], in_=woutb[i * 128:(i + 1) * 128, :])
            bout_sb = sing.tile([128, CT], f32, tag="bout", name="bout")
            nc.scalar.dma_start(out=bout_sb[:], in_=boutT[:])
            maskf_sb = sing.tile([128, NT], f32, tag="maskf", name="maskf")
            nc.scalar.dma_start(out=maskf_sb[:], in_=maskf[:])
            m8q_sb = sing.tile([128, NT], f32, tag="m8q", name="m8q")
            nc.scalar.dma_start(out=m8q_sb[:], in_=m8q[:])
            tempP_sb = sing.tile([128, CT], f32, tag="tempP", name="tempP")
            nc.scalar.dma_start(out=tempP_sb[:], in_=tempP[:])
            ident_sb = sing.tile([128, 128], bf16, tag="ident", name="ident")
            nc.scalar.dma_start(out=ident_sb[:], in_=identb[:])

            ones1 = sing.tile([128, 1], bf16, tag="ones1", name="ones1")
            nc.vector.memset(ones1[:], 1.0)
            onesR = sing.tile([1, 128], bf16, tag="onesR", name="onesR")
            nc.vector.memset(onesR[:], 1.0)

            nrm = sing.tile([128, CT, NCH], f32, tag="nrm", name="nrm")
            pi_all = sing.tile([128, NCH, TPC, H], bf16, tag="pi_all", name="pi_all")
            amat = [sing.tile([128, 16], bf16, tag=f"amat{i}", name=f"amat{i}") for i in range(CT)]
            for i in range(CT):
                nc.scalar.dma_start(out=amat[i][:], in_=amatc[:, i * 16:(i + 1) * 16])
            invt = sing.tile([128, CT], f32, tag="invt", name="invt")
            spr = sing.tile([1, 16], f32, tag="spr", name="spr")
            watn = sing.tile([1, H], f32, tag="watn", name="watn")
            wexf = sing.tile([128, H], f32, tag="wexf", name="wexf")

            def phase1(first):
                with (
                    tc.tile_pool(name="p1x", bufs=8) as xp,
                    tc.tile_pool(name="p1ps", bufs=6, space="PSUM") as psp,
                ):
                    # PE clock warmup while the first DMAs land
                    warm = xp.tile([128, 512], bf16, tag="warm", name="warm")
                    nc.vector.memset(warm[:], 0.0)
                    psW = psp.tile([1, 512], f32, tag="psA", name="psW")
                    for i in range(6):
                        nc.tensor.matmul(psW[:], ones1[:], warm[:],
                                         start=(i == 0), stop=(i == 5))
                    for k in range(NCH):
                        xt = []
                        for ci in range(CT):
                            if first and k == 0:
                                nc.sync.dma_start(
                                    out=wq[ci][:],
                                    in_=wqkvb[ci * 128:(ci + 1) * 128, :])
                            t = xp.tile([128, 512], bf16, tag="xt", name="xt")
                            nc.sync.dma_start(
                                out=t[:],
                                in_=xbf[ci * 128:(ci + 1) * 128, k * 512:(k + 1) * 512])
                            xt.append(t)
                        for co in range(CT):
                            psA = psp.tile([128, 512], f32, tag="psA", name="psA")
                            for ci in range(CT):
                                nc.tensor.matmul(
                                    psA[:], wq[ci][:, co * 128:(co + 1) * 128], xt[ci][:],
                                    start=(ci == 0), stop=(ci == CT - 1))
                            nc.scalar.activation(
                                out=wt[co][:, k * 512:(k + 1) * 512], in_=psA[:],
                                func=Act.Copy)
                            # w2 = psA^2 (bf16) and norm^2 partial in one DVE pass
                            nc.vector.tensor_tensor_reduce(
                                out=w2[co][:, k * 512:(k + 1) * 512],
                                in0=psA[:], in1=psA[:], scale=1.0, scalar=0.0,
                                op0=Alu.mult, op1=Alu.add,
                                accum_out=nrm[:, co, k:k + 1])

            def norm_finalize():
                # amat inv columns, batched over all 4 channel tiles
                nsq = small.tile([128, CT], f32, tag="nsq_f", name="nsq_f")
                nc.vector.reduce_sum(nsq[:], nrm[:], axis=mybir.AxisListType.X)
                nc.vector.tensor_scalar_max(nsq[:], nsq[:], 1e-24)
                nc.vector.reciprocal(nsq[:], nsq[:])
                nc.vector.tensor_mul(invt[:], nsq[:], tempP_sb[:])
                for ci in range(CT):
                    nc.vector.tensor_copy(
                        amat[ci][0:64, 2 * ci:2 * ci + 1], invt[0:64, ci:ci + 1])
                    nc.vector.tensor_copy(
                        amat[ci][64:128, 2 * ci + 1:2 * ci + 2], invt[64:128, ci:ci + 1])

            def phase2():
                with (
                    tc.tile_pool(name="p2w", bufs=3) as wp,
                    tc.tile_pool(name="p2psB", bufs=3, space="PSUM") as psb,
                    tc.tile_pool(name="p2psT", bufs=2, space="PSUM") as pst,
                    tc.tile_pool(name="p2psS", bufs=1, space="PSUM") as pss,
                ):
                    psS = pss.tile([1, 2 * TPC * H], f32, tag="psS", name="psS")
                    for k in range(NCH):
                        # one PSUM bank holds all 4 token tiles' (sum_ws | r)
                        pb = psb.tile([128, TPC, 16], f32, tag="psB", name="psB")
                        for ti in range(TPC):
                            t = k * TPC + ti
                            for ci in range(CT):
                                nc.tensor.matmul(
                                    pb[:, ti, :], w2[ci][:, t * 128:(t + 1) * 128],
                                    amat[ci][:],
                                    start=(ci == 0), stop=(ci == CT - 1))
                        # head softmax, token-major; masked tokens blend to the
                        # exact uniform 1/8 via erec*mask and +(1-mask)/8
                        ee = wp.tile([128, TPC, H], f32, tag="ee", name="ee")
                        nc.scalar.activation(out=ee[:], in_=pb[:, :, 0:H], func=Act.Exp)
                        erec = wp.tile([128, TPC], f32, tag="erec", name="erec")
                        nc.vector.reduce_sum(erec[:], ee[:], axis=mybir.AxisListType.X)
                        nc.vector.reciprocal(erec[:], erec[:])
                        nc.vector.tensor_mul(
                            erec[:], erec[:], maskf_sb[:, k * TPC:(k + 1) * TPC])
                        for ti in range(TPC):
                            t = k * TPC + ti
                            nc.vector.tensor_scalar(
                                out=pi_all[:, k, ti, :], in0=ee[:, ti, :],
                                scalar1=erec[:, ti:ti + 1], scalar2=m8q_sb[:, t:t + 1],
                                op0=Alu.mult, op1=Alu.add)
                        pirt = wp.tile([128, TPC, H], bf16, tag="pirt", name="pirt")
                        nc.vector.tensor_mul(
                            pirt[:], pi_all[:, k, :, :], pb[:, :, 8:16])
                        # global S / PR accumulators: one bank, two column groups
                        nc.tensor.matmul(
                            psS[0:1, 0:TPC * H], ones1[:], pi_all[:, k, :, :],
                            start=(k == 0), stop=(k == NCH - 1))
                        nc.tensor.matmul(
                            psS[0:1, TPC * H:2 * TPC * H], ones1[:], pirt[:],
                            start=(k == 0), stop=(k == NCH - 1))
                        # Pi back to channel layout: PE transpose, then 0-stride
                        # broadcast DMAs straight from PSUM (SP + Act queues)
                        psT = pst.tile([H, 512], bf16, tag="psT", name="psT")
                        for ti in range(TPC):
                            nc.tensor.transpose(
                                psT[:, ti * 128:(ti + 1) * 128],
                                pi_all[:, k, ti, :], ident_sb[:])
                        pitc = wp.tile([H, 512], bf16, tag="pitc", name="pitc")
                        nc.scalar.activation(out=pitc[:], in_=psT[:], func=Act.Copy)
                        for ci in range(CT):
                            eng = nc.sync if ci < 3 else nc.scalar
                            veng = nc.vector if ci % 2 == 0 else nc.gpsimd
                            pexp = wp.tile([128, 512], bf16, tag=f"pexp{ci}",
                                           name=f"pexp{ci}")
                            src = pitc[2 * ci:2 * ci + 2, :]
                            bsrc = bass.AP(tensor=src.tensor, offset=src.offset,
                                           ap=[src.ap[0], [0, 64], src.ap[1]])
                            eng.dma_start(out=pexp[:], in_=bsrc)
                            veng.tensor_mul(
                                wt[ci][:, k * 512:(k + 1) * 512],
                                wt[ci][:, k * 512:(k + 1) * 512], pexp[:])
                    # spr[0,0:8] = S[h], spr[0,8:16] = PR[h] (read psS before
                    # the pool scope releases the bank)
                    nc.vector.reduce_sum(
                        spr[:].rearrange("p (g h) -> p g h", g=2),
                        psS[:].rearrange("p (g t h) -> p g h t", g=2, t=TPC, h=H),
                        axis=mybir.AxisListType.X)

            def global_scalars():
                # -attn = -(S+eps) / (S+eps+PR), broadcast to all partitions
                # with a ones-matmul, folded into the wo weights per parity
                with tc.tile_pool(name="gs", bufs=1, space="PSUM") as psg:
                    s0 = small.tile([1, H], f32, tag="s0", name="s0")
                    nc.vector.tensor_scalar_add(s0[:], spr[0:1, 0:H], 1e-8)
                    s1 = small.tile([1, H], f32, tag="s1", name="s1")
                    nc.vector.tensor_add(s1[:], s0[:], spr[0:1, H:2 * H])
                    nc.vector.reciprocal(s1[:], s1[:])
                    nc.vector.scalar_tensor_tensor(
                        out=watn[:], in0=s0[:], scalar=-1.0, in1=s1[:],
                        op0=Alu.mult, op1=Alu.mult)
                    watnb = small.tile([1, H], bf16, tag="watnb", name="watnb")
                    nc.vector.tensor_copy(watnb[:], watn[:])
                    psG = psg.tile([128, H], f32, tag="psG", name="psG")
                    nc.tensor.matmul(psG[:], onesR[:], watnb[:], start=True, stop=True)
                    nc.scalar.activation(out=wexf[:], in_=psG[:], func=Act.Copy)
                    for ci in range(CT):
                        veng = nc.vector if ci % 2 == 0 else nc.gpsimd
                        veng.tensor_scalar_mul(
                            wo[ci][0:64, :], wo[ci][0:64, :],
                            wexf[0:64, 2 * ci:2 * ci + 1])
                        veng.tensor_scalar_mul(
                            wo[ci][64:128, :], wo[ci][64:128, :],
                            wexf[64:128, 2 * ci + 1:2 * ci + 2])

            def phase3():
                with (
                    tc.tile_pool(name="p3o", bufs=6) as op,
                    tc.tile_pool(name="p3ps", bufs=6, space="PSUM") as psp,
                ):
                    for k in range(NCH):
                        for oj in range(CT):
                            psC = psp.tile([128, 512], f32, tag="psC", name="psC")
                            for ci in range(CT):
                                nc.tensor.matmul(
                                    psC[:], wo[ci][:, oj * 128:(oj + 1) * 128],
                                    wt[ci][:, k * 512:(k + 1) * 512],
                                    start=(ci == 0), stop=(ci == CT - 1))
                            oc = op.tile([128, 512], bf16, tag="outc", name="outc")
                            if oj % 2 == 0:
                                nc.scalar.activation(
                                    out=oc[:], in_=psC[:], func=Act.Identity,
                                    bias=bout_sb[:, oj:oj + 1], scale=1.0)
                            else:
                                nc.vector.tensor_scalar_add(
                                    oc[:], psC[:], bout_sb[:, oj:oj + 1])
                            # drain the last chunk on both DMA queues
                            deng = nc.scalar if (k == NCH - 1 and oj >= 2) else nc.sync
                            deng.dma_start(
                                out=outT[oj * 128:(oj + 1) * 128, k * 512:(k + 1) * 512],
                                in_=oc[:])

            for _rep in range(reps):
                if 1 in phases:
                    phase1(first=(_rep == 0))
                    norm_finalize()
                if 2 in phases:
                    phase2()
                if 3 in phases:
                    global_scalars()
                    phase3()

    nc.compile()
    return nc


def _prep_inputs(x, token_mask, Wqkv, temp, Wout, bout):
    import ml_dtypes
    f = np.float32
    bf = ml_dtypes.bfloat16
    temp = np.asarray(temp, dtype=f)
    wqkvb = np.ascontiguousarray(np.asarray(Wqkv, f).T.astype(bf))
    woutb = np.ascontiguousarray(np.asarray(Wout, f).T.astype(bf))
    boutT = np.ascontiguousarray(np.asarray(bout, f).reshape(CT, 128).T)
    identb = np.eye(128, dtype=bf)
    # amat template: indicator columns 8..15 filled, inv columns 0..7 zero
    amatc = np.zeros((128, CT, 16), np.float32)
    for ci in range(CT):
        amatc[0:64, ci, 8 + 2 * ci] = 1.0
        amatc[64:128, ci, 8 + 2 * ci + 1] = 1.0
    amatc = np.ascontiguousarray(amatc.reshape(128, CT * 16).astype(bf))
    # tempP[p, ci] = temp[2ci + (p>=64)]
    tempP = np.empty((128, CT), f)
    for ci in range(CT):
        tempP[0:64, ci] = temp[2 * ci, 0]
        tempP[64:128, ci] = temp[2 * ci + 1, 0]
    in_maps = []
    for b in range(B):
        m = np.asarray(token_mask[b], f)          # [N]
        mt = m.reshape(NT, 128).T.copy()          # [128, NT]
        in_maps.append({
            "xbf": np.ascontiguousarray(np.asarray(x[b], f).T.astype(bf)),
            "wqkvb": wqkvb,
            "woutb": woutb,
            "boutT": boutT,
            "maskf": mt,
            "m8q": np.ascontiguousarray((1.0 - mt) / 8.0),
            "tempP": tempP,
            "identb": identb,
            "amatc": amatc,
        })
    return in_maps


def kernel(**inputs):
    from concourse.bass_utils import run_bass_kernel_spmd

    if "nc" not in _CACHE:
        _CACHE["nc"] = _build_bass()
    nc = _CACHE["nc"]
    in_maps = _prep_inputs(**inputs)
    try:
        res = run_bass_kernel_spmd(nc, in_maps, core_ids=list(range(B)))
    except Exception:
        # transient device/tunnel hiccup: retry once
        import time as _t
        _t.sleep(2.0)
        res = run_bass_kernel_spmd(nc, in_maps, core_ids=list(range(B)))
    out = np.empty((B, N, C), np.float32)
    for b in range(B):
        out[b] = res.results[b]["outT"].T.astype(np.float32)
    return out
